# revision 11
# baseline (speedup 1.0000x reference)
"""Causal GQA attention (B=2,T=2048,D=1024,H=16,KV=4) on 8 trn2 cores.

Sharding: core = b*4 + g  (batch b, kv-group g).  Each core computes the
4 query heads of its group for its batch plus the row-parallel partial of
the output projection; the host sums the 4 partials per batch.

v2: fully software-pipelined emission — input DMA overlaps projections,
projection/out-projection matmuls are interleaved as fillers into the
attention loop so the PE never idles (keeps the HAM p-state warm), causal
masks run on gpsimd, 1/l normalization is broadcast via swdge partition
broadcast, and y partials are written back in bf16.
"""

import os
import numpy as np
import ml_dtypes

import concourse.bass as bass
import concourse.tile as tile
import concourse.mybir as mybir
from concourse import bacc
from concourse.bass_utils import run_bass_kernel_spmd
from concourse.masks import make_identity

F32 = mybir.dt.float32
BF16 = mybir.dt.bfloat16
AF = mybir.ActivationFunctionType

B, T, C, HEADS, KVH, HD = 2, 2048, 1024, 16, 4, 64
G = HEADS // KVH          # 4 query heads per kv group
DG = G * HD               # 256 columns per group
NCORES = 8
SCALE = 1.0 / 8.0         # 1/sqrt(HD)
NT = T // 512             # 4 q-blocks of 512
NKT = T // 128            # 16 k-tiles of 128

SWAP = []
for _i in range(16):
    SWAP += [2 * _i + 1, 2 * _i]

_CACHE = {}
LAST_EXEC_NS = None
LAST_PROFILE = None


def _install_trace_hook():
    import sys, types
    try:
        import antenv.axon_hooks  # noqa: F401
        return
    except ImportError:
        pass
    try:
        from trn_agent_boot.trn_boot import _ntff_profile_via_ctypes
        hook = _ntff_profile_via_ctypes('/opt/axon/libaxon_pjrt.so')
    except Exception:
        hook = None
    mod = types.ModuleType('antenv.axon_hooks')
    mod.get_axon_ntff_profile_hook = lambda: hook
    mod.set_axon_ntff_profile_hook = lambda h: None
    sys.modules['antenv.axon_hooks'] = mod


def _build(debug=False):
    nc = bacc.Bacc("TRN2", target_bir_lowering=False, debug=debug)

    xT_d = nc.dram_tensor("xT", [C, T], BF16, kind="ExternalInput")
    sin2t_d = nc.dram_tensor("sin2t", [128, T], BF16, kind="ExternalInput")
    cos2t_d = nc.dram_tensor("cos2t", [128, T], BF16, kind="ExternalInput")
    wq_d = nc.dram_tensor("wq", [C, DG], BF16, kind="ExternalInput")
    wkv_d = nc.dram_tensor("wkv", [C, 128], BF16, kind="ExternalInput")
    wo_d = nc.dram_tensor("wo", [DG, C], BF16, kind="ExternalInput")
    y_d = nc.dram_tensor("y", [T, C], BF16, kind="ExternalOutput")

    with tile.TileContext(nc) as tc:
        with (
            tc.tile_pool(name="persist", bufs=1) as persist,
            tc.tile_pool(name="stage", bufs=3) as stage,
            tc.tile_pool(name="ps", bufs=2, space="PSUM") as ps,
            tc.tile_pool(name="pc", bufs=1, space="PSUM") as pc,
            tc.tile_pool(name="pj", bufs=2, space="PSUM") as pj,
            tc.tile_pool(name="pr", bufs=1, space="PSUM") as pr,
        ):
            # pin the sp tag to the low psum banks
            heater = ps.tile([128, 1024], F32, tag="sp")
            nc.vector.memset(heater[0:1, 0:8], 0.0)

            # ---- persistent SBUF tiles ----
            id64b = persist.tile([64, 64], BF16, tag="id64b")
            make_identity(nc, id64b[:])
            mskab = persist.tile([128, 2, 1024], BF16, tag="mskab")
            wqbf = persist.tile([128, 8, DG], BF16, tag="wqbf")
            wkvbf = persist.tile([128, 8, 128], BF16, tag="wkvbf")
            wobf = persist.tile([128, 2, C], BF16, tag="wobf")
            sin2t = persist.tile([128, T], BF16, tag="sin2t")
            cos2t = persist.tile([128, T], BF16, tag="cos2t")
            xtbf = persist.tile([128, 8, T], BF16, tag="xtbf")
            khat = persist.tile([64, T], BF16, tag="khat")
            qhat = [persist.tile([128, T], BF16, tag=f"qhat{m}", name=f"qhat{m}") for m in range(2)]
            qodd = [persist.tile([64, T], BF16, tag=f"qodd{m}", name=f"qodd{m}") for m in range(2)]
            vtbf = persist.tile([64, T], BF16, tag="vtbf")
            vp = persist.tile([128, NKT, HD + 1], BF16, tag="vp")
            ctxn = [persist.tile([128, T], BF16, tag=f"ctxn{m}", name=f"ctxn{m}") for m in range(2)]
            l16 = persist.tile([4, NT, 512], F32, tag="l16")
            r16 = persist.tile([4, NT, 512], F32, tag="r16")
            rscr = persist.tile([4, 512], F32, tag="rscr")
            rbf = persist.tile([4, NT, 512], BF16, tag="rbf")
            rcb = persist.tile([4, 512], F32, tag="rcb")
            rres = persist.tile([4, 512], F32, tag="rres")
            rres_bf = persist.tile([4, NT, 512], BF16, tag="rres_bf")
            e4 = persist.tile([4, 4, 64], BF16, tag="e4")
            nc.gpsimd.memset(e4[:], 0.0)
            nc.gpsimd.affine_select(
                out=e4[:], in_=e4[:],
                compare_op=mybir.AluOpType.not_equal,
                fill=1.0, base=0,
                pattern=[[-1, 4], [0, 64]],
                channel_multiplier=1)

            # ---- input DMAs ----
            # scalar queue: constants + weights (x goes on sync queue);
            # sin/cos first (RoPE is the earliest consumer after x)
            nc.scalar.dma_start(out=sin2t[:], in_=sin2t_d[:, :])
            nc.scalar.dma_start(out=cos2t[:], in_=cos2t_d[:, :])
            nc.scalar.dma_start(out=wqbf[:], in_=wq_d.rearrange("(c p) d -> p c d", p=128))
            nc.scalar.dma_start(out=wobf[:], in_=wo_d.rearrange("(m p) e -> p m e", p=128))
            # causal masks generated on-device (gpsimd affine_select):
            # mskab[p, q, c] covers the 4 diagonal k-tiles (offsets 0..384):
            # msk[p, c] = ((c - p - off) >= 0)
            nc.gpsimd.memset(mskab[:], 1.0)
            for qi, off in enumerate((0, 128, 256, 384)):
                nc.gpsimd.affine_select(
                    out=mskab[:, qi // 2, (qi % 2) * 512:(qi % 2 + 1) * 512],
                    in_=mskab[:, qi // 2, (qi % 2) * 512:(qi % 2 + 1) * 512],
                    compare_op=mybir.AluOpType.is_ge,
                    fill=0.0, base=-off,
                    pattern=[[1, 512]],
                    channel_multiplier=-1)
            # preload the exp table while DMAs stream
            warm = stage.tile([1, 8], F32, tag="warm")
            nc.scalar.activation(warm[:], e4[0:1, 0, 0:8], AF.Exp,
                                 bias=0.0, scale=1.0)

            # sync queue: wkv first, then x in (t-half, ct) chunk order so
            # tb=0/1 projections can start early
            nc.sync.dma_start(out=wkvbf[:],
                              in_=wkv_d.rearrange("(c p) d -> p c d", p=128))
            for th in range(2):
                tsl = slice(th * 1024, (th + 1) * 1024)
                for ct in range(8):
                    nc.sync.dma_start(out=xtbf[:, ct, tsl],
                                      in_=xT_d[ct * 128:(ct + 1) * 128, tsl])

            nc.vector.memset(vp[:, :, HD:HD + 1], 1.0)

            # ---- emission helpers ----
            def kv_steps(tb):
                """Closures: packed K|V projection + RoPE(K) + V transpose."""
                ts_ = slice(tb * 512, (tb + 1) * 512)
                pkv = [None]

                def mk_mm(ct):
                    def f():
                        if ct == 0:
                            pkv[0] = pj.tile([128, 512], F32, tag="pp", name="pkv")
                        nc.tensor.matmul(pkv[0][:], wkvbf[:, ct, :],
                                         xtbf[:, ct, ts_],
                                         start=(ct == 0), stop=(ct == 7))
                    return f

                def evicts():
                    nc.vector.tensor_copy(out=vtbf[:, ts_], in_=pkv[0][64:128, :])
                    ksh = stage.tile([64, 512], F32, tag="ksh")
                    nc.vector.stream_shuffle(ksh[:], pkv[0][0:64, :], SWAP)
                    t1 = stage.tile([64, 512], F32, tag="t1k")
                    nc.vector.tensor_mul(t1[:], pkv[0][0:64, :], cos2t[0:64, ts_])
                    t2 = stage.tile([64, 512], F32, tag="t2k")
                    nc.vector.tensor_mul(t2[:], ksh[:], sin2t[0:64, ts_])
                    nc.vector.tensor_add(khat[:, ts_], t1[:], t2[:])

                def transposes():
                    for k4 in range(4):
                        kt = tb * 4 + k4
                        vt = pr.tile([128, HD], BF16, tag="vt")
                        nc.tensor.transpose(
                            vt[:], vtbf[:, kt * 128:(kt + 1) * 128], id64b[:])
                        nc.vector.tensor_copy(out=vp[:, kt, 0:HD], in_=vt[:])

                return [mk_mm(ct) for ct in range(8)] + [evicts, transposes]

            def q_steps(tb, m):
                """Closures: Q pair projection + RoPE(Q) (+ qodd copy)."""
                ts_ = slice(tb * 512, (tb + 1) * 512)
                pq = [None]

                def mk_mm(ct):
                    def f():
                        if ct == 0:
                            pq[0] = pj.tile([128, 512], F32, tag="pp", name="pq")
                        nc.tensor.matmul(pq[0][:],
                                         wqbf[:, ct, m * 128:(m + 1) * 128],
                                         xtbf[:, ct, ts_],
                                         start=(ct == 0), stop=(ct == 7))
                    return f

                def evicts():
                    qsh = stage.tile([128, 512], F32, tag="qsh", bufs=2)
                    nc.vector.stream_shuffle(qsh[:], pq[0][:], SWAP)
                    t1 = stage.tile([128, 512], F32, tag="t1q", bufs=2)
                    nc.vector.tensor_mul(t1[:], pq[0][:], cos2t[:, ts_])
                    t2 = stage.tile([128, 512], F32, tag="t2q", bufs=2)
                    nc.vector.tensor_mul(t2[:], qsh[:], sin2t[:, ts_])
                    nc.vector.tensor_add(qhat[m][:, ts_], t1[:], t2[:])
                    nc.scalar.dma_start(out=qodd[m][:, ts_],
                                        in_=qhat[m][64:128, ts_])

                return [mk_mm(ct) for ct in range(8)] + [evicts]

            def outproj_steps(qb):
                steps = []
                for ti in range(4):
                    tt = qb * 4 + ti

                    def f(tt=tt):
                        ysb = stage.tile([128, C], BF16, tag="ysb", bufs=3)
                        for eb in range(2):
                            yp = pj.tile([128, 512], F32, tag="pp")
                            for mi in range(2):
                                nc.tensor.matmul(
                                    yp[:], ctxn[mi][:, tt * 128:(tt + 1) * 128],
                                    wobf[:, mi, eb * 512:(eb + 1) * 512],
                                    start=(mi == 0), stop=(mi == 1))
                            nc.vector.tensor_copy(
                                out=ysb[:, eb * 512:(eb + 1) * 512], in_=yp[:])
                        nc.sync.dma_start(out=y_d[tt * 128:(tt + 1) * 128, :],
                                          in_=ysb[:])
                    steps.append(f)
                return steps

            fillers = []   # list of (tag, closure)

            def pop_fill(k):
                for _ in range(min(k, len(fillers))):
                    fillers.pop(0)[1]()

            def flush_fill():
                while fillers:
                    fillers.pop(0)[1]()

            def flush_until(tag):
                while any(t == tag for t, _ in fillers):
                    fillers.pop(0)[1]()

            def tag_steps(tag, steps):
                return [(tag, s) for s in steps]

            def emit_attn(qb):
                qs_ = slice(qb * 512, (qb + 1) * 512)
                kt_last = 4 * qb + 3
                cxq = stage.tile([65, 4, 512], F32, tag="cxq", bufs=2)
                for h in range(G):
                    m, lo = divmod(h, 2)
                    qrhs = qhat[m] if lo == 0 else qodd[m]
                    ctx = pc.tile([65, 512], F32, tag="ctx")

                    def mm_pair(sp, kt0):
                        for half in range(2):
                            kt = kt0 + half
                            nc.tensor.matmul(
                                sp[:, half * 512:(half + 1) * 512],
                                khat[:, kt * 128:(kt + 1) * 128],
                                qrhs[0:64, qs_],
                                start=True, stop=True)

                    # diagonal k-tiles first: scores + exp into a merged
                    # [128, 2048] pbf, one mask multiply; their ctx MMs are
                    # deferred to the end of the head so the mask latency
                    # hides behind the off-diagonal work.
                    pbfd = stage.tile([128, 2, 1024], BF16, tag="pbfd", bufs=2)
                    for dpi in range(2):
                        sp = ps.tile([128, 1024], F32, tag="sp")
                        mm_pair(sp, 4 * qb + 2 * dpi)
                        nc.scalar.activation(pbfd[:, dpi, :], sp[:], AF.Exp,
                                             bias=0.0, scale=SCALE)
                    nc.vector.tensor_mul(pbfd[:], pbfd[:], mskab[:])

                    # off-diagonal pi loop with 2-deep score lookahead
                    n_off = 2 * qb

                    def emit_sp(pi):
                        sp = ps.tile([128, 1024], F32, tag="sp")
                        mm_pair(sp, 2 * pi)
                        pbf = stage.tile([128, 1024], BF16, tag="pbf", bufs=6)
                        nc.scalar.activation(pbf[:], sp[:], AF.Exp,
                                             bias=0.0, scale=SCALE)
                        return pbf

                    pbfs = {}
                    for pi in range(min(2, n_off)):
                        pbfs[pi] = emit_sp(pi)
                    for pi in range(n_off):
                        pop_fill(1)
                        pbf = pbfs.pop(pi)
                        for half in range(2):
                            kt = 2 * pi + half
                            nc.tensor.matmul(
                                ctx[:], vp[:, kt, :],
                                pbf[:, half * 512:(half + 1) * 512],
                                start=(kt == 0), stop=False)
                        if pi + 2 < n_off:
                            pbfs[pi + 2] = emit_sp(pi + 2)
                    # diagonal ctx MMs last
                    pop_fill(4 if qb == 0 else 2)
                    for j in range(4):
                        kt = 4 * qb + j
                        nc.tensor.matmul(
                            ctx[:], vp[:, kt, :],
                            pbfd[:, j // 2, (j % 2) * 512:(j % 2 + 1) * 512],
                            start=(kt == 0), stop=(kt == kt_last))
                    nc.vector.tensor_copy(out=cxq[:, h, :], in_=ctx[:, :])
                    nc.sync.dma_start(out=l16[h:h + 1, qb, :],
                                      in_=cxq[64:65, h, :])
                return cxq

            def normalize_steps(qb, cxq):
                qs_ = slice(qb * 512, (qb + 1) * 512)

                def rchain():
                    nc.vector.reciprocal_approx_accurate(
                        r16[:, qb, :], l16[:, qb, :], rscr[:])
                    nc.vector.tensor_copy(out=rbf[:, qb, :], in_=r16[:, qb, :])
                    nc.vector.tensor_copy(out=rcb[:], in_=rbf[:, qb, :])
                    nc.vector.tensor_sub(rres[:], r16[:, qb, :], rcb[:])
                    nc.vector.tensor_copy(out=rres_bf[:, qb, :], in_=rres[:])

                steps = [rchain]
                for h in range(G):
                    m, lo = divmod(h, 2)

                    def nh(h=h, m=m, lo=lo):
                        bc = pj.tile([128, 512], F32, tag="pp", name="bc")
                        nc.tensor.matmul(bc[0:64, :], e4[:, h, :],
                                         rbf[:, qb, :], start=True, stop=False)
                        nc.tensor.matmul(bc[0:64, :], e4[:, h, :],
                                         rres_bf[:, qb, :],
                                         start=False, stop=True)
                        cn = stage.tile([64, 512], BF16, tag="cn", bufs=2)
                        nc.vector.tensor_mul(cn[:], cxq[0:64, h, :],
                                             bc[0:64, :])
                        nc.sync.dma_start(
                            out=ctxn[m][lo * 64:(lo + 1) * 64, qs_], in_=cn[:])

                    steps.append(nh)
                return steps

            # ---- the schedule ----
            # load phase: tb=0/1 projections as a block (PE trickles while x
            # streams in).  Attention order 1,2,3,0: the tail out-projection
            # belongs to the smallest block.  Projections/normalize/out-
            # projection interleave into attention as fillers.
            for f in (kv_steps(0) + q_steps(0, 0) + q_steps(0, 1)
                      + kv_steps(1) + q_steps(1, 0) + q_steps(1, 1)):
                f()

            fillers.extend(tag_steps("p2", kv_steps(2) + q_steps(2, 0)
                                     + q_steps(2, 1)))
            cx = emit_attn(1)
            fillers.extend(tag_steps("n1", normalize_steps(1, cx)))
            fillers.extend(tag_steps("o1", outproj_steps(1)))
            fillers.extend(tag_steps("p3", kv_steps(3) + q_steps(3, 0)
                                     + q_steps(3, 1)))
            flush_until("p2")
            cx = emit_attn(2)
            fillers.extend(tag_steps("n2", normalize_steps(2, cx)))
            fillers.extend(tag_steps("o2", outproj_steps(2)))
            flush_until("p3")
            cx = emit_attn(3)
            fillers.extend(tag_steps("n3", normalize_steps(3, cx)))
            fillers.extend(tag_steps("o3", outproj_steps(3)))
            cx = emit_attn(0)
            fillers.extend(tag_steps("n0", normalize_steps(0, cx)))
            fillers.extend(tag_steps("o0", outproj_steps(0)))
            flush_fill()

    nc.compile()
    return nc


def kernel(x, sin, cos, mask, Wq, Wk, Wv, Wo):
    global LAST_EXEC_NS, LAST_PROFILE
    if "nc" not in _CACHE:
        _CACHE["nc"] = _build()
    nc = _CACHE["nc"]

    x = np.asarray(x, np.float32)
    sin = np.asarray(sin, np.float32)
    cos = np.asarray(cos, np.float32)
    Wq, Wk, Wv, Wo = (np.asarray(w, np.float32) for w in (Wq, Wk, Wv, Wo))

    sinT = np.ascontiguousarray(sin.T)            # [64, T]
    sin2t = np.concatenate([sinT, sinT], axis=0).copy()
    sin2t[0::2, :] *= -1.0  # fold rot sign: rot = sign * pair-swap
    sin2t = sin2t.astype(ml_dtypes.bfloat16)
    cosT = np.ascontiguousarray(cos.T)
    cos2t = np.concatenate([cosT, cosT], axis=0).astype(ml_dtypes.bfloat16)

    in_maps = []
    for core in range(NCORES):
        b, g = divmod(core, KVH)
        wkv = np.concatenate(
            [Wk[:, g * HD:(g + 1) * HD], Wv[:, g * HD:(g + 1) * HD]], axis=1)
        in_maps.append({
            "xT": np.ascontiguousarray(x[b].T).astype(ml_dtypes.bfloat16),
            "sin2t": sin2t,
            "cos2t": cos2t,
            "wq": np.ascontiguousarray(Wq[:, g * DG:(g + 1) * DG]).astype(ml_dtypes.bfloat16),
            "wkv": np.ascontiguousarray(wkv).astype(ml_dtypes.bfloat16),
            "wo": np.ascontiguousarray(Wo[g * DG:(g + 1) * DG, :]).astype(ml_dtypes.bfloat16),
        })

    trace = os.environ.get("KERNEL_TRACE", "0") == "1"
    if trace:
        _install_trace_hook()
    res = run_bass_kernel_spmd(nc, in_maps, core_ids=list(range(NCORES)),
                               trace=trace)
    LAST_EXEC_NS = res.exec_time_ns
    LAST_PROFILE = res.profile_json

    y = np.zeros((B, T, C), np.float32)
    for core in range(NCORES):
        b = core // KVH
        y[b] += res.results[core]["y"].astype(np.float32)
    return y


# revision 12
# speedup vs baseline: 1.0490x; 1.0490x over previous
"""Causal GQA attention (B=2,T=2048,D=1024,H=16,KV=4) on 8 trn2 cores.

Sharding: core = b*4 + g  (batch b, kv-group g).  Each core computes the
4 query heads of its group for its batch plus the row-parallel partial of
the output projection; the host sums the 4 partials per batch.

v2: fully software-pipelined emission — input DMA overlaps projections,
projection/out-projection matmuls are interleaved as fillers into the
attention loop so the PE never idles (keeps the HAM p-state warm), causal
masks run on gpsimd, 1/l normalization is broadcast via swdge partition
broadcast, and y partials are written back in bf16.
"""

import os
import numpy as np
import ml_dtypes

import concourse.bass as bass
import concourse.tile as tile
import concourse.mybir as mybir
from concourse import bacc
from concourse.bass_utils import run_bass_kernel_spmd
from concourse.masks import make_identity

F32 = mybir.dt.float32
BF16 = mybir.dt.bfloat16
AF = mybir.ActivationFunctionType

B, T, C, HEADS, KVH, HD = 2, 2048, 1024, 16, 4, 64
G = HEADS // KVH          # 4 query heads per kv group
DG = G * HD               # 256 columns per group
NCORES = 8
SCALE = 1.0 / 8.0         # 1/sqrt(HD)
NT = T // 512             # 4 q-blocks of 512
NKT = T // 128            # 16 k-tiles of 128

SWAP = []
for _i in range(16):
    SWAP += [2 * _i + 1, 2 * _i]

_CACHE = {}
LAST_EXEC_NS = None
LAST_PROFILE = None


def _install_trace_hook():
    import sys, types
    try:
        import antenv.axon_hooks  # noqa: F401
        return
    except ImportError:
        pass
    try:
        from trn_agent_boot.trn_boot import _ntff_profile_via_ctypes
        hook = _ntff_profile_via_ctypes('/opt/axon/libaxon_pjrt.so')
    except Exception:
        hook = None
    mod = types.ModuleType('antenv.axon_hooks')
    mod.get_axon_ntff_profile_hook = lambda: hook
    mod.set_axon_ntff_profile_hook = lambda h: None
    sys.modules['antenv.axon_hooks'] = mod


def _build(debug=False):
    nc = bacc.Bacc("TRN2", target_bir_lowering=False, debug=debug)

    xT_d = nc.dram_tensor("xT", [C, T], BF16, kind="ExternalInput")
    sin2t_d = nc.dram_tensor("sin2t", [128, T], BF16, kind="ExternalInput")
    cos2t_d = nc.dram_tensor("cos2t", [128, T], BF16, kind="ExternalInput")
    wq_d = nc.dram_tensor("wq", [C, DG], BF16, kind="ExternalInput")
    wkv_d = nc.dram_tensor("wkv", [C, 128], BF16, kind="ExternalInput")
    wo_d = nc.dram_tensor("wo", [DG, C], BF16, kind="ExternalInput")
    y_d = nc.dram_tensor("y", [T, C], BF16, kind="ExternalOutput")

    with tile.TileContext(nc) as tc:
        with (
            tc.tile_pool(name="persist", bufs=1) as persist,
            tc.tile_pool(name="stage", bufs=3) as stage,
            tc.tile_pool(name="ps", bufs=2, space="PSUM") as ps,
            tc.tile_pool(name="pc", bufs=1, space="PSUM") as pc,
            tc.tile_pool(name="pj", bufs=2, space="PSUM") as pj,
            tc.tile_pool(name="pr", bufs=1, space="PSUM") as pr,
        ):
            # pin the sp tag to the low psum banks
            heater = ps.tile([128, 1024], F32, tag="sp")
            nc.vector.memset(heater[0:1, 0:8], 0.0)

            # ---- persistent SBUF tiles ----
            id64b = persist.tile([64, 64], BF16, tag="id64b")
            make_identity(nc, id64b[:])
            mskab = persist.tile([128, 2, 1024], BF16, tag="mskab")
            wqbf = persist.tile([128, 8, DG], BF16, tag="wqbf")
            wkvbf = persist.tile([128, 8, 128], BF16, tag="wkvbf")
            wobf = persist.tile([128, 2, C], BF16, tag="wobf")
            sin2t = persist.tile([128, T], BF16, tag="sin2t")
            cos2t = persist.tile([128, T], BF16, tag="cos2t")
            xtbf = persist.tile([128, 8, T], BF16, tag="xtbf")
            khat = persist.tile([64, T], BF16, tag="khat")
            qhat = [persist.tile([128, T], BF16, tag=f"qhat{m}", name=f"qhat{m}") for m in range(2)]
            qodd = [persist.tile([64, T], BF16, tag=f"qodd{m}", name=f"qodd{m}") for m in range(2)]
            vtbf = persist.tile([64, T], BF16, tag="vtbf")
            vp = persist.tile([128, NKT, HD + 1], BF16, tag="vp")
            ctxn = [persist.tile([128, T], BF16, tag=f"ctxn{m}", name=f"ctxn{m}") for m in range(2)]
            l16 = persist.tile([4, NT, 512], F32, tag="l16")
            r16 = persist.tile([4, NT, 512], F32, tag="r16")
            rscr = persist.tile([4, 512], F32, tag="rscr")
            rbf = persist.tile([4, NT, 512], BF16, tag="rbf")
            rcb = persist.tile([4, 512], F32, tag="rcb")
            rres = persist.tile([4, 512], F32, tag="rres")
            rres_bf = persist.tile([4, NT, 512], BF16, tag="rres_bf")
            e4 = persist.tile([4, 4, 64], BF16, tag="e4")
            nc.gpsimd.memset(e4[:], 0.0)
            nc.gpsimd.affine_select(
                out=e4[:], in_=e4[:],
                compare_op=mybir.AluOpType.not_equal,
                fill=1.0, base=0,
                pattern=[[-1, 4], [0, 64]],
                channel_multiplier=1)

            # ---- input DMAs ----
            # scalar queue: constants + weights (x goes on sync queue);
            # sin/cos first (RoPE is the earliest consumer after x)
            nc.scalar.dma_start(out=sin2t[:], in_=sin2t_d[:, :])
            nc.scalar.dma_start(out=cos2t[:], in_=cos2t_d[:, :])
            nc.scalar.dma_start(out=wqbf[:], in_=wq_d.rearrange("(c p) d -> p c d", p=128))
            nc.scalar.dma_start(out=wobf[:], in_=wo_d.rearrange("(m p) e -> p m e", p=128))
            # causal masks generated on-device (gpsimd affine_select):
            # mskab[p, q, c] covers the 4 diagonal k-tiles (offsets 0..384):
            # msk[p, c] = ((c - p - off) >= 0)
            nc.gpsimd.memset(mskab[:], 1.0)
            for qi, off in enumerate((0, 128, 256, 384)):
                nc.gpsimd.affine_select(
                    out=mskab[:, qi // 2, (qi % 2) * 512:(qi % 2 + 1) * 512],
                    in_=mskab[:, qi // 2, (qi % 2) * 512:(qi % 2 + 1) * 512],
                    compare_op=mybir.AluOpType.is_ge,
                    fill=0.0, base=-off,
                    pattern=[[1, 512]],
                    channel_multiplier=-1)
            # preload the exp table while DMAs stream
            warm = stage.tile([1, 8], F32, tag="warm")
            nc.scalar.activation(warm[:], e4[0:1, 0, 0:8], AF.Exp,
                                 bias=0.0, scale=1.0)

            # sync queue: wkv first, then x in (t-half, ct) chunk order so
            # tb=0/1 projections can start early
            nc.sync.dma_start(out=wkvbf[:],
                              in_=wkv_d.rearrange("(c p) d -> p c d", p=128))
            for th in range(2):
                tsl = slice(th * 1024, (th + 1) * 1024)
                for ct in range(8):
                    nc.sync.dma_start(out=xtbf[:, ct, tsl],
                                      in_=xT_d[ct * 128:(ct + 1) * 128, tsl])

            nc.vector.memset(vp[:, :, HD:HD + 1], 1.0)

            # ---- emission helpers ----
            def kv_evict(tb, pkv):
                ts_ = slice(tb * 512, (tb + 1) * 512)
                nc.vector.tensor_copy(out=vtbf[:, ts_], in_=pkv[64:128, :])
                ksh = stage.tile([64, 512], F32, tag="ksh")
                nc.vector.stream_shuffle(ksh[:], pkv[0:64, :], SWAP)
                t1 = stage.tile([64, 512], F32, tag="t1k")
                nc.vector.tensor_mul(t1[:], pkv[0:64, :], cos2t[0:64, ts_])
                t2 = stage.tile([64, 512], F32, tag="t2k")
                nc.vector.tensor_mul(t2[:], ksh[:], sin2t[0:64, ts_])
                nc.vector.tensor_add(khat[:, ts_], t1[:], t2[:])

            def kv_transposes(tb):
                for k4 in range(4):
                    kt = tb * 4 + k4
                    vt = pr.tile([128, HD], BF16, tag="vt")
                    nc.tensor.transpose(
                        vt[:], vtbf[:, kt * 128:(kt + 1) * 128], id64b[:])
                    nc.vector.tensor_copy(out=vp[:, kt, 0:HD], in_=vt[:])

            def q_evict(tb, m, pq):
                ts_ = slice(tb * 512, (tb + 1) * 512)
                qsh = stage.tile([128, 512], F32, tag="qsh", bufs=2)
                nc.vector.stream_shuffle(qsh[:], pq[:], SWAP)
                t1 = stage.tile([128, 512], F32, tag="t1q", bufs=2)
                nc.vector.tensor_mul(t1[:], pq[:], cos2t[:, ts_])
                t2 = stage.tile([128, 512], F32, tag="t2q", bufs=2)
                nc.vector.tensor_mul(t2[:], qsh[:], sin2t[:, ts_])
                nc.vector.tensor_add(qhat[m][:, ts_], t1[:], t2[:])
                nc.scalar.dma_start(out=qodd[m][:, ts_],
                                    in_=qhat[m][64:128, ts_])

            def kv_steps(tb):
                """Closures: packed K|V projection + RoPE(K) + V transpose."""
                ts_ = slice(tb * 512, (tb + 1) * 512)
                pkv = [None]

                def mk_mm(ct):
                    def f():
                        if ct == 0:
                            pkv[0] = pj.tile([128, 512], F32, tag="pp", name="pkv")
                        nc.tensor.matmul(pkv[0][:], wkvbf[:, ct, :],
                                         xtbf[:, ct, ts_],
                                         start=(ct == 0), stop=(ct == 7))
                    return f

                return ([mk_mm(ct) for ct in range(8)]
                        + [lambda: kv_evict(tb, pkv[0][:]),
                           lambda: kv_transposes(tb)])

            def q_steps(tb, m):
                """Closures: Q pair projection + RoPE(Q) (+ qodd copy)."""
                ts_ = slice(tb * 512, (tb + 1) * 512)
                pq = [None]

                def mk_mm(ct):
                    def f():
                        if ct == 0:
                            pq[0] = pj.tile([128, 512], F32, tag="pp", name="pq")
                        nc.tensor.matmul(pq[0][:],
                                         wqbf[:, ct, m * 128:(m + 1) * 128],
                                         xtbf[:, ct, ts_],
                                         start=(ct == 0), stop=(ct == 7))
                    return f

                return [mk_mm(ct) for ct in range(8)] + [lambda: q_evict(tb, m, pq[0][:])]

            def outproj_steps(qb):
                steps = []
                for ti in range(4):
                    tt = qb * 4 + ti

                    def f(tt=tt):
                        ysb = stage.tile([128, C], BF16, tag="ysb", bufs=3)
                        for eb in range(2):
                            yp = pj.tile([128, 512], F32, tag="pp")
                            for mi in range(2):
                                nc.tensor.matmul(
                                    yp[:], ctxn[mi][:, tt * 128:(tt + 1) * 128],
                                    wobf[:, mi, eb * 512:(eb + 1) * 512],
                                    start=(mi == 0), stop=(mi == 1))
                            nc.vector.tensor_copy(
                                out=ysb[:, eb * 512:(eb + 1) * 512], in_=yp[:])
                        nc.sync.dma_start(out=y_d[tt * 128:(tt + 1) * 128, :],
                                          in_=ysb[:])
                    steps.append(f)
                return steps

            fillers = []   # list of (tag, closure)

            def pop_fill(k):
                for _ in range(min(k, len(fillers))):
                    fillers.pop(0)[1]()

            def flush_fill():
                while fillers:
                    fillers.pop(0)[1]()

            def flush_until(tag):
                while any(t == tag for t, _ in fillers):
                    fillers.pop(0)[1]()

            def tag_steps(tag, steps):
                return [(tag, s) for s in steps]

            def emit_attn(qb):
                qs_ = slice(qb * 512, (qb + 1) * 512)
                n_pi = 2 * (qb + 1)
                kt_last = 4 * qb + 3
                cxq = stage.tile([65, 4, 512], F32, tag="cxq", bufs=2)
                for h in range(G):
                    m, lo = divmod(h, 2)
                    qrhs = qhat[m] if lo == 0 else qodd[m]
                    ctx = pc.tile([65, 512], F32, tag="ctx")

                    def emit_sp(pi):
                        sp = ps.tile([128, 1024], F32, tag="sp")
                        for half in range(2):
                            kt = 2 * pi + half
                            nc.tensor.matmul(
                                sp[:, half * 512:(half + 1) * 512],
                                khat[:, kt * 128:(kt + 1) * 128],
                                qrhs[0:64, qs_],
                                start=True, stop=True)
                        pbf = stage.tile([128, 1024], BF16, tag="pbf", bufs=6)
                        nc.scalar.activation(pbf[:], sp[:], AF.Exp,
                                             bias=0.0, scale=SCALE)
                        if pi == 2 * qb:
                            dmsk = mskab[:, 0, :]
                        elif pi == 2 * qb + 1:
                            dmsk = mskab[:, 1, :]
                        else:
                            dmsk = None
                        if dmsk is not None:
                            nc.vector.tensor_mul(pbf[:, 0:512], pbf[:, 0:512],
                                                 dmsk[:, 0:512])
                            nc.vector.tensor_mul(pbf[:, 512:1024],
                                                 pbf[:, 512:1024],
                                                 dmsk[:, 512:1024])
                        return pbf

                    pbfs = {0: emit_sp(0)}
                    if n_pi > 1:
                        pbfs[1] = emit_sp(1)
                    for pi in range(n_pi):
                        pop_fill(4 if pi >= 2 * qb else 1)
                        pbf = pbfs.pop(pi)
                        for half in range(2):
                            kt = 2 * pi + half
                            nc.tensor.matmul(
                                ctx[:], vp[:, kt, :],
                                pbf[:, half * 512:(half + 1) * 512],
                                start=(kt == 0), stop=(kt == kt_last))
                        if pi + 2 < n_pi:
                            pbfs[pi + 2] = emit_sp(pi + 2)
                    nc.vector.tensor_copy(out=cxq[:, h, :], in_=ctx[:, :])
                    nc.sync.dma_start(out=l16[h:h + 1, qb, :],
                                      in_=cxq[64:65, h, :])
                return cxq

            def normalize_steps(qb, cxq):
                qs_ = slice(qb * 512, (qb + 1) * 512)

                def rchain():
                    nc.vector.reciprocal_approx_accurate(
                        r16[:, qb, :], l16[:, qb, :], rscr[:])
                    nc.vector.tensor_copy(out=rbf[:, qb, :], in_=r16[:, qb, :])
                    nc.vector.tensor_copy(out=rcb[:], in_=rbf[:, qb, :])
                    nc.vector.tensor_sub(rres[:], r16[:, qb, :], rcb[:])
                    nc.vector.tensor_copy(out=rres_bf[:, qb, :], in_=rres[:])

                steps = [rchain]
                for h in range(G):
                    m, lo = divmod(h, 2)

                    def nh(h=h, m=m, lo=lo):
                        bc = pj.tile([128, 512], F32, tag="pp", name="bc")
                        nc.tensor.matmul(bc[0:64, :], e4[:, h, :],
                                         rbf[:, qb, :], start=True, stop=False)
                        nc.tensor.matmul(bc[0:64, :], e4[:, h, :],
                                         rres_bf[:, qb, :],
                                         start=False, stop=True)
                        cn = stage.tile([64, 512], BF16, tag="cn", bufs=2)
                        nc.vector.tensor_mul(cn[:], cxq[0:64, h, :],
                                             bc[0:64, :])
                        nc.sync.dma_start(
                            out=ctxn[m][lo * 64:(lo + 1) * 64, qs_], in_=cn[:])

                    steps.append(nh)
                return steps

            # ---- the schedule ----
            # load phase: tb=0/1 projections interleaved per x-chunk so each
            # chunk is fully consumed on arrival (PE keeps pace with the DMA
            # stream).  Attention order 1,2,3,0: the tail out-projection
            # belongs to the smallest block.
            spA = ps.tile([128, 1024], F32, tag="sp", name="lpa")
            spB = ps.tile([128, 1024], F32, tag="sp", name="lpb")
            ppA = pj.tile([128, 512], F32, tag="pp", name="lpc")
            ppB = pj.tile([128, 512], F32, tag="pp", name="lpd")
            lchains = [
                ("kv", 0, spA[:, 0:512]),
                ("q", (0, 0), spA[:, 512:1024]),
                ("q", (0, 1), spB[:, 0:512]),
                ("kv", 1, spB[:, 512:1024]),
                ("q", (1, 0), ppA[:]),
                ("q", (1, 1), ppB[:]),
            ]
            for ct in range(8):
                for kind, key, out_ap in lchains:
                    if kind == "kv":
                        ts_ = slice(key * 512, (key + 1) * 512)
                        nc.tensor.matmul(out_ap, wkvbf[:, ct, :],
                                         xtbf[:, ct, ts_],
                                         start=(ct == 0), stop=(ct == 7))
                    else:
                        tb, m = key
                        ts_ = slice(tb * 512, (tb + 1) * 512)
                        nc.tensor.matmul(out_ap,
                                         wqbf[:, ct, m * 128:(m + 1) * 128],
                                         xtbf[:, ct, ts_],
                                         start=(ct == 0), stop=(ct == 7))
            kv_evict(0, spA[:, 0:512])
            kv_transposes(0)
            q_evict(0, 0, spA[:, 512:1024])
            q_evict(0, 1, spB[:, 0:512])
            kv_evict(1, spB[:, 512:1024])
            kv_transposes(1)
            q_evict(1, 0, ppA[:])
            q_evict(1, 1, ppB[:])

            fillers.extend(tag_steps("p2", kv_steps(2) + q_steps(2, 0)
                                     + q_steps(2, 1)))
            cx = emit_attn(1)
            fillers.extend(tag_steps("n1", normalize_steps(1, cx)))
            fillers.extend(tag_steps("o1", outproj_steps(1)))
            fillers.extend(tag_steps("p3", kv_steps(3) + q_steps(3, 0)
                                     + q_steps(3, 1)))
            flush_until("p2")
            cx = emit_attn(2)
            fillers.extend(tag_steps("n2", normalize_steps(2, cx)))
            fillers.extend(tag_steps("o2", outproj_steps(2)))
            flush_until("p3")
            cx = emit_attn(3)
            fillers.extend(tag_steps("n3", normalize_steps(3, cx)))
            fillers.extend(tag_steps("o3", outproj_steps(3)))
            cx = emit_attn(0)
            fillers.extend(tag_steps("n0", normalize_steps(0, cx)))
            fillers.extend(tag_steps("o0", outproj_steps(0)))
            flush_fill()

    nc.compile()
    return nc


def kernel(x, sin, cos, mask, Wq, Wk, Wv, Wo):
    global LAST_EXEC_NS, LAST_PROFILE
    if "nc" not in _CACHE:
        _CACHE["nc"] = _build()
    nc = _CACHE["nc"]

    x = np.asarray(x, np.float32)
    sin = np.asarray(sin, np.float32)
    cos = np.asarray(cos, np.float32)
    Wq, Wk, Wv, Wo = (np.asarray(w, np.float32) for w in (Wq, Wk, Wv, Wo))

    sinT = np.ascontiguousarray(sin.T)            # [64, T]
    sin2t = np.concatenate([sinT, sinT], axis=0).copy()
    sin2t[0::2, :] *= -1.0  # fold rot sign: rot = sign * pair-swap
    sin2t = sin2t.astype(ml_dtypes.bfloat16)
    cosT = np.ascontiguousarray(cos.T)
    cos2t = np.concatenate([cosT, cosT], axis=0).astype(ml_dtypes.bfloat16)

    in_maps = []
    for core in range(NCORES):
        b, g = divmod(core, KVH)
        wkv = np.concatenate(
            [Wk[:, g * HD:(g + 1) * HD], Wv[:, g * HD:(g + 1) * HD]], axis=1)
        in_maps.append({
            "xT": np.ascontiguousarray(x[b].T).astype(ml_dtypes.bfloat16),
            "sin2t": sin2t,
            "cos2t": cos2t,
            "wq": np.ascontiguousarray(Wq[:, g * DG:(g + 1) * DG]).astype(ml_dtypes.bfloat16),
            "wkv": np.ascontiguousarray(wkv).astype(ml_dtypes.bfloat16),
            "wo": np.ascontiguousarray(Wo[g * DG:(g + 1) * DG, :]).astype(ml_dtypes.bfloat16),
        })

    trace = os.environ.get("KERNEL_TRACE", "0") == "1"
    if trace:
        _install_trace_hook()
    res = run_bass_kernel_spmd(nc, in_maps, core_ids=list(range(NCORES)),
                               trace=trace)
    LAST_EXEC_NS = res.exec_time_ns
    LAST_PROFILE = res.profile_json

    y = np.zeros((B, T, C), np.float32)
    for core in range(NCORES):
        b = core // KVH
        y[b] += res.results[core]["y"].astype(np.float32)
    return y


# revision 14
# speedup vs baseline: 1.0685x; 1.0186x over previous
"""Causal GQA attention (B=2,T=2048,D=1024,H=16,KV=4) on 8 trn2 cores.

Sharding: core = b*4 + g  (batch b, kv-group g).  Each core computes the
4 query heads of its group for its batch plus the row-parallel partial of
the output projection; the host sums the 4 partials per batch.

v2: fully software-pipelined emission — input DMA overlaps projections,
projection/out-projection matmuls are interleaved as fillers into the
attention loop so the PE never idles (keeps the HAM p-state warm), causal
masks run on gpsimd, 1/l normalization is broadcast via swdge partition
broadcast, and y partials are written back in bf16.
"""

import os
import numpy as np
import ml_dtypes

import concourse.bass as bass
import concourse.tile as tile
import concourse.mybir as mybir
from concourse import bacc
from concourse.bass_utils import run_bass_kernel_spmd
from concourse.masks import make_identity

F32 = mybir.dt.float32
BF16 = mybir.dt.bfloat16
AF = mybir.ActivationFunctionType

B, T, C, HEADS, KVH, HD = 2, 2048, 1024, 16, 4, 64
G = HEADS // KVH          # 4 query heads per kv group
DG = G * HD               # 256 columns per group
NCORES = 8
SCALE = 1.0 / 8.0         # 1/sqrt(HD)
NT = T // 512             # 4 q-blocks of 512
NKT = T // 128            # 16 k-tiles of 128

SWAP = []
for _i in range(16):
    SWAP += [2 * _i + 1, 2 * _i]

_CACHE = {}
LAST_EXEC_NS = None
LAST_PROFILE = None


def _install_trace_hook():
    import sys, types
    try:
        import antenv.axon_hooks  # noqa: F401
        return
    except ImportError:
        pass
    try:
        from trn_agent_boot.trn_boot import _ntff_profile_via_ctypes
        hook = _ntff_profile_via_ctypes('/opt/axon/libaxon_pjrt.so')
    except Exception:
        hook = None
    mod = types.ModuleType('antenv.axon_hooks')
    mod.get_axon_ntff_profile_hook = lambda: hook
    mod.set_axon_ntff_profile_hook = lambda h: None
    sys.modules['antenv.axon_hooks'] = mod


def _build(debug=False):
    nc = bacc.Bacc("TRN2", target_bir_lowering=False, debug=debug)

    xT_d = nc.dram_tensor("xT", [C, T], BF16, kind="ExternalInput")
    sin2t_d = nc.dram_tensor("sin2t", [128, T], BF16, kind="ExternalInput")
    cos2t_d = nc.dram_tensor("cos2t", [128, T], BF16, kind="ExternalInput")
    wq_d = nc.dram_tensor("wq", [C, DG], BF16, kind="ExternalInput")
    wkv_d = nc.dram_tensor("wkv", [C, 128], BF16, kind="ExternalInput")
    wo_d = nc.dram_tensor("wo", [DG, C], BF16, kind="ExternalInput")
    y_d = nc.dram_tensor("y", [T, C], BF16, kind="ExternalOutput")

    with tile.TileContext(nc) as tc:
        with (
            tc.tile_pool(name="persist", bufs=1) as persist,
            tc.tile_pool(name="stage", bufs=3) as stage,
            tc.tile_pool(name="ps", bufs=2, space="PSUM") as ps,
            tc.tile_pool(name="pc", bufs=1, space="PSUM") as pc,
            tc.tile_pool(name="pj", bufs=2, space="PSUM") as pj,
            tc.tile_pool(name="pr", bufs=1, space="PSUM") as pr,
        ):
            # pin the sp tag to the low psum banks
            heater = ps.tile([128, 1024], F32, tag="sp")
            nc.vector.memset(heater[0:1, 0:8], 0.0)

            # ---- persistent SBUF tiles ----
            id64b = persist.tile([64, 64], BF16, tag="id64b")
            make_identity(nc, id64b[:])
            mskab = persist.tile([128, 2, 1024], BF16, tag="mskab")
            wqbf = persist.tile([128, 8, DG], BF16, tag="wqbf")
            wkvbf = persist.tile([128, 8, 128], BF16, tag="wkvbf")
            wobf = persist.tile([128, 2, C], BF16, tag="wobf")
            sin2t = persist.tile([128, T], BF16, tag="sin2t")
            cos2t = persist.tile([128, T], BF16, tag="cos2t")
            x_t = [[persist.tile([128, 1024], BF16, tag=f"x{th}_{ct}",
                                 name=f"x{th}_{ct}")
                    for ct in range(8)] for th in range(2)]
            khat = persist.tile([64, T], BF16, tag="khat")
            qhat = [persist.tile([128, T], BF16, tag=f"qhat{m}", name=f"qhat{m}") for m in range(2)]
            qodd = [persist.tile([64, T], BF16, tag=f"qodd{m}", name=f"qodd{m}") for m in range(2)]
            vtbf = persist.tile([64, T], BF16, tag="vtbf")
            vp = persist.tile([128, NKT, HD + 1], BF16, tag="vp")
            ctxn = [persist.tile([128, T], BF16, tag=f"ctxn{m}", name=f"ctxn{m}") for m in range(2)]
            l16 = persist.tile([4, NT, 512], F32, tag="l16")
            r16 = persist.tile([4, NT, 512], F32, tag="r16")
            rscr = persist.tile([4, 512], F32, tag="rscr")
            rbf = persist.tile([4, NT, 512], BF16, tag="rbf")
            rcb = persist.tile([4, 512], F32, tag="rcb")
            rres = persist.tile([4, 512], F32, tag="rres")
            rres_bf = persist.tile([4, NT, 512], BF16, tag="rres_bf")
            e4 = persist.tile([4, 4, 64], BF16, tag="e4")
            nc.gpsimd.memset(e4[:], 0.0)
            nc.gpsimd.affine_select(
                out=e4[:], in_=e4[:],
                compare_op=mybir.AluOpType.not_equal,
                fill=1.0, base=0,
                pattern=[[-1, 4], [0, 64]],
                channel_multiplier=1)

            # ---- input DMAs ----
            # scalar queue: constants + weights (x goes on sync queue);
            # sin/cos first (RoPE is the earliest consumer after x)
            nc.scalar.dma_start(out=sin2t[:], in_=sin2t_d[:, :])
            nc.scalar.dma_start(out=cos2t[:], in_=cos2t_d[:, :])
            nc.scalar.dma_start(out=wqbf[:], in_=wq_d.rearrange("(c p) d -> p c d", p=128))
            nc.scalar.dma_start(out=wobf[:], in_=wo_d.rearrange("(m p) e -> p m e", p=128))
            # causal masks generated on-device (gpsimd affine_select):
            # mskab[p, q, c] covers the 4 diagonal k-tiles (offsets 0..384):
            # msk[p, c] = ((c - p - off) >= 0)
            nc.gpsimd.memset(mskab[:], 1.0)
            for qi, off in enumerate((0, 128, 256, 384)):
                nc.gpsimd.affine_select(
                    out=mskab[:, qi // 2, (qi % 2) * 512:(qi % 2 + 1) * 512],
                    in_=mskab[:, qi // 2, (qi % 2) * 512:(qi % 2 + 1) * 512],
                    compare_op=mybir.AluOpType.is_ge,
                    fill=0.0, base=-off,
                    pattern=[[1, 512]],
                    channel_multiplier=-1)
            # preload the exp table while DMAs stream
            warm = stage.tile([1, 8], F32, tag="warm")
            nc.scalar.activation(warm[:], e4[0:1, 0, 0:8], AF.Exp,
                                 bias=0.0, scale=1.0)

            # sync queue: wkv first, then x in (t-half, ct) chunk order so
            # tb=0/1 projections can start early
            nc.sync.dma_start(out=wkvbf[:],
                              in_=wkv_d.rearrange("(c p) d -> p c d", p=128))
            for th in range(2):
                tsl = slice(th * 1024, (th + 1) * 1024)
                for ct in range(8):
                    nc.sync.dma_start(out=x_t[th][ct][:],
                                      in_=xT_d[ct * 128:(ct + 1) * 128, tsl])

            def xs(tb, ct):
                th, half = divmod(tb, 2)
                return x_t[th][ct][:, half * 512:(half + 1) * 512]

            nc.vector.memset(vp[:, :, HD:HD + 1], 1.0)

            # ---- emission helpers ----
            def kv_evict(tb, pkv):
                ts_ = slice(tb * 512, (tb + 1) * 512)
                nc.vector.tensor_copy(out=vtbf[:, ts_], in_=pkv[64:128, :])
                ksh = stage.tile([64, 512], F32, tag="ksh")
                nc.vector.stream_shuffle(ksh[:], pkv[0:64, :], SWAP)
                t1 = stage.tile([64, 512], F32, tag="t1k")
                nc.vector.tensor_mul(t1[:], pkv[0:64, :], cos2t[0:64, ts_])
                t2 = stage.tile([64, 512], F32, tag="t2k")
                nc.vector.tensor_mul(t2[:], ksh[:], sin2t[0:64, ts_])
                nc.vector.tensor_add(khat[:, ts_], t1[:], t2[:])

            def kv_transposes(tb):
                for k4 in range(4):
                    kt = tb * 4 + k4
                    vt = pr.tile([128, HD], BF16, tag="vt")
                    nc.tensor.transpose(
                        vt[:], vtbf[:, kt * 128:(kt + 1) * 128], id64b[:])
                    nc.vector.tensor_copy(out=vp[:, kt, 0:HD], in_=vt[:])

            def q_evict(tb, m, pq):
                ts_ = slice(tb * 512, (tb + 1) * 512)
                qsh = stage.tile([128, 512], F32, tag="qsh", bufs=2)
                nc.vector.stream_shuffle(qsh[:], pq[:], SWAP)
                t1 = stage.tile([128, 512], F32, tag="t1q", bufs=2)
                nc.vector.tensor_mul(t1[:], pq[:], cos2t[:, ts_])
                t2 = stage.tile([128, 512], F32, tag="t2q", bufs=2)
                nc.vector.tensor_mul(t2[:], qsh[:], sin2t[:, ts_])
                nc.vector.tensor_add(qhat[m][:, ts_], t1[:], t2[:])
                nc.scalar.dma_start(out=qodd[m][:, ts_],
                                    in_=qhat[m][64:128, ts_])

            def q_steps(tb, m):
                """Closures: Q pair projection + RoPE(Q) (+ qodd copy)."""
                ts_ = slice(tb * 512, (tb + 1) * 512)
                pq = [None]

                def mk_mm(ct):
                    def f():
                        if ct == 0:
                            pq[0] = pj.tile([128, 512], F32, tag="pp", name="pq")
                        nc.tensor.matmul(pq[0][:],
                                         wqbf[:, ct, m * 128:(m + 1) * 128],
                                         xs(tb, ct),
                                         start=(ct == 0), stop=(ct == 7))
                    return f

                return [mk_mm(ct) for ct in range(8)] + [lambda: q_evict(tb, m, pq[0][:])]

            def outproj_steps(qb, tail=False):
                steps = []
                for ti in range(4):
                    tt = qb * 4 + ti

                    def f(tt=tt):
                        ysb = stage.tile([128, C], BF16, tag="ysb", bufs=3)
                        for eb in range(2):
                            yp = pj.tile([128, 512], F32, tag="pp")
                            for mi in range(2):
                                nc.tensor.matmul(
                                    yp[:], ctxn[mi][:, tt * 128:(tt + 1) * 128],
                                    wobf[:, mi, eb * 512:(eb + 1) * 512],
                                    start=(mi == 0), stop=(mi == 1))
                            if tail and eb == 1:
                                nc.scalar.copy(
                                    out=ysb[:, eb * 512:(eb + 1) * 512],
                                    in_=yp[:])
                            else:
                                nc.vector.tensor_copy(
                                    out=ysb[:, eb * 512:(eb + 1) * 512],
                                    in_=yp[:])
                        nc.sync.dma_start(out=y_d[tt * 128:(tt + 1) * 128, :],
                                          in_=ysb[:])
                    steps.append(f)
                return steps

            fillers = []   # list of (tag, closure)

            def pop_fill(k):
                for _ in range(min(k, len(fillers))):
                    fillers.pop(0)[1]()

            def flush_fill():
                while fillers:
                    fillers.pop(0)[1]()

            def flush_until(tag):
                while any(t == tag for t, _ in fillers):
                    fillers.pop(0)[1]()

            def tag_steps(tag, steps):
                return [(tag, s) for s in steps]

            def emit_attn(qb):
                qs_ = slice(qb * 512, (qb + 1) * 512)
                n_pi = 2 * (qb + 1)
                kt_last = 4 * qb + 3
                cxq = stage.tile([65, 4, 512], F32, tag="cxq", bufs=2)
                slots = [(h, pi) for h in range(G) for pi in range(n_pi)]
                pbfs = {}
                ctxs = {}

                def emit_sp(h, pi):
                    m, lo = divmod(h, 2)
                    qrhs = qhat[m] if lo == 0 else qodd[m]
                    sp = ps.tile([128, 1024], F32, tag="sp")
                    for half in range(2):
                        kt = 2 * pi + half
                        nc.tensor.matmul(
                            sp[:, half * 512:(half + 1) * 512],
                            khat[:, kt * 128:(kt + 1) * 128],
                            qrhs[0:64, qs_],
                            start=True, stop=True)
                    pbf = stage.tile([128, 1024], BF16, tag="pbf", bufs=6)
                    nc.scalar.activation(pbf[:], sp[:], AF.Exp,
                                         bias=0.0, scale=SCALE)
                    if pi == 2 * qb:
                        dmsk = mskab[:, 0, :]
                    elif pi == 2 * qb + 1:
                        dmsk = mskab[:, 1, :]
                    else:
                        dmsk = None
                    if dmsk is not None:
                        nc.vector.tensor_mul(pbf[:, 0:512], pbf[:, 0:512],
                                             dmsk[:, 0:512])
                        nc.vector.tensor_mul(pbf[:, 512:1024],
                                             pbf[:, 512:1024],
                                             dmsk[:, 512:1024])
                    pbfs[(h, pi)] = pbf

                for s in range(min(2, len(slots))):
                    emit_sp(*slots[s])
                for s, (h, pi) in enumerate(slots):
                    if s + 2 < len(slots):
                        emit_sp(*slots[s + 2])
                    pop_fill(3 if pi >= 2 * qb else 1)
                    if pi == 0:
                        ctxs[h] = pc.tile([65, 512], F32, tag="ctx", name="ctx")
                    ctx = ctxs[h]
                    pbf = pbfs.pop((h, pi))
                    for half in range(2):
                        kt = 2 * pi + half
                        nc.tensor.matmul(
                            ctx[:], vp[:, kt, :],
                            pbf[:, half * 512:(half + 1) * 512],
                            start=(kt == 0), stop=(kt == kt_last))
                    if pi == n_pi - 1:
                        nc.vector.tensor_copy(out=cxq[:, h, :], in_=ctx[:, :])
                        nc.sync.dma_start(out=l16[h:h + 1, qb, :],
                                          in_=cxq[64:65, h, :])
                return cxq

            def normalize_steps(qb, cxq):
                qs_ = slice(qb * 512, (qb + 1) * 512)

                def rchain():
                    nc.vector.reciprocal_approx_accurate(
                        r16[:, qb, :], l16[:, qb, :], rscr[:])
                    nc.vector.tensor_copy(out=rbf[:, qb, :], in_=r16[:, qb, :])
                    nc.vector.tensor_copy(out=rcb[:], in_=rbf[:, qb, :])
                    nc.vector.tensor_sub(rres[:], r16[:, qb, :], rcb[:])
                    nc.vector.tensor_copy(out=rres_bf[:, qb, :], in_=rres[:])

                steps = [rchain]
                for h in range(G):
                    m, lo = divmod(h, 2)

                    def nh(h=h, m=m, lo=lo):
                        bc = pj.tile([128, 512], F32, tag="pp", name="bc")
                        nc.tensor.matmul(bc[0:64, :], e4[:, h, :],
                                         rbf[:, qb, :], start=True, stop=False)
                        nc.tensor.matmul(bc[0:64, :], e4[:, h, :],
                                         rres_bf[:, qb, :],
                                         start=False, stop=True)
                        cn = stage.tile([64, 512], BF16, tag="cn", bufs=2)
                        nc.vector.tensor_mul(cn[:], cxq[0:64, h, :],
                                             bc[0:64, :])
                        nc.sync.dma_start(
                            out=ctxn[m][lo * 64:(lo + 1) * 64, qs_], in_=cn[:])

                    steps.append(nh)
                return steps

            # ---- the schedule ----
            # load: kv0/kv1/q1* interleaved per th0-chunk, kv2/kv3 per
            # th1-chunk; q0* deferred (attention runs 1,2,3,0 so tb=0 queries
            # are needed last).  Everything else flows in as fillers.
            spA = ps.tile([128, 1024], F32, tag="sp", name="lpa")
            spB = ps.tile([128, 1024], F32, tag="sp", name="lpb")
            ppA = pj.tile([128, 512], F32, tag="pp", name="lpc")
            ppB = pj.tile([128, 512], F32, tag="pp", name="lpd")
            for ct in range(8):
                nc.tensor.matmul(spA[:, 0:512], wkvbf[:, ct, :], xs(0, ct),
                                 start=(ct == 0), stop=(ct == 7))
                nc.tensor.matmul(spA[:, 512:1024], wkvbf[:, ct, :], xs(1, ct),
                                 start=(ct == 0), stop=(ct == 7))
                nc.tensor.matmul(spB[:, 0:512], wqbf[:, ct, 0:128], xs(1, ct),
                                 start=(ct == 0), stop=(ct == 7))
                nc.tensor.matmul(spB[:, 512:1024], wqbf[:, ct, 128:256],
                                 xs(1, ct), start=(ct == 0), stop=(ct == 7))
            kv_evict(0, spA[:, 0:512])
            kv_evict(1, spA[:, 512:1024])
            q_evict(1, 0, spB[:, 0:512])
            q_evict(1, 1, spB[:, 512:1024])
            kv_transposes(0)
            kv_transposes(1)
            for ct in range(8):
                nc.tensor.matmul(ppA[:], wkvbf[:, ct, :], xs(2, ct),
                                 start=(ct == 0), stop=(ct == 7))
                nc.tensor.matmul(ppB[:], wkvbf[:, ct, :], xs(3, ct),
                                 start=(ct == 0), stop=(ct == 7))
            kv_evict(2, ppA[:])
            kv_evict(3, ppB[:])
            kv_transposes(2)
            kv_transposes(3)

            fillers.extend(tag_steps("p2", q_steps(2, 0) + q_steps(2, 1)))
            cx = emit_attn(1)
            fillers.extend(tag_steps("n1", normalize_steps(1, cx)))
            fillers.extend(tag_steps("o1", outproj_steps(1)))
            fillers.extend(tag_steps("p3", q_steps(3, 0) + q_steps(3, 1)))
            flush_until("p2")
            cx = emit_attn(2)
            fillers.extend(tag_steps("n2", normalize_steps(2, cx)))
            fillers.extend(tag_steps("o2", outproj_steps(2)))
            fillers.extend(tag_steps("p0", q_steps(0, 0) + q_steps(0, 1)))
            flush_until("p3")
            cx = emit_attn(3)
            fillers.extend(tag_steps("n3", normalize_steps(3, cx)))
            fillers.extend(tag_steps("o3", outproj_steps(3)))
            flush_until("p0")
            cx = emit_attn(0)
            fillers.extend(tag_steps("n0", normalize_steps(0, cx)))
            fillers.extend(tag_steps("o0", outproj_steps(0, tail=True)))
            flush_fill()

    nc.compile()
    return nc


def kernel(x, sin, cos, mask, Wq, Wk, Wv, Wo):
    global LAST_EXEC_NS, LAST_PROFILE
    if "nc" not in _CACHE:
        _CACHE["nc"] = _build()
    nc = _CACHE["nc"]

    x = np.asarray(x, np.float32)
    sin = np.asarray(sin, np.float32)
    cos = np.asarray(cos, np.float32)
    Wq, Wk, Wv, Wo = (np.asarray(w, np.float32) for w in (Wq, Wk, Wv, Wo))

    sinT = np.ascontiguousarray(sin.T)            # [64, T]
    sin2t = np.concatenate([sinT, sinT], axis=0).copy()
    sin2t[0::2, :] *= -1.0  # fold rot sign: rot = sign * pair-swap
    sin2t = sin2t.astype(ml_dtypes.bfloat16)
    cosT = np.ascontiguousarray(cos.T)
    cos2t = np.concatenate([cosT, cosT], axis=0).astype(ml_dtypes.bfloat16)

    in_maps = []
    for core in range(NCORES):
        b, g = divmod(core, KVH)
        wkv = np.concatenate(
            [Wk[:, g * HD:(g + 1) * HD], Wv[:, g * HD:(g + 1) * HD]], axis=1)
        in_maps.append({
            "xT": np.ascontiguousarray(x[b].T).astype(ml_dtypes.bfloat16),
            "sin2t": sin2t,
            "cos2t": cos2t,
            "wq": np.ascontiguousarray(Wq[:, g * DG:(g + 1) * DG]).astype(ml_dtypes.bfloat16),
            "wkv": np.ascontiguousarray(wkv).astype(ml_dtypes.bfloat16),
            "wo": np.ascontiguousarray(Wo[g * DG:(g + 1) * DG, :]).astype(ml_dtypes.bfloat16),
        })

    trace = os.environ.get("KERNEL_TRACE", "0") == "1"
    if trace:
        _install_trace_hook()
    res = run_bass_kernel_spmd(nc, in_maps, core_ids=list(range(NCORES)),
                               trace=trace)
    LAST_EXEC_NS = res.exec_time_ns
    LAST_PROFILE = res.profile_json

    y = np.zeros((B, T, C), np.float32)
    for core in range(NCORES):
        b = core // KVH
        y[b] += res.results[core]["y"].astype(np.float32)
    return y
